# revision 7
# baseline (speedup 1.0000x reference)
"""Trainium2 Bass kernel for ParallelLMHeadWithLoRA.

logits = hidden @ W^T + (hidden @ A^T) @ B^T
  hidden [2048, 4096] f32, W [32000, 4096] f32, A [16, 4096], B [32000, 16]

Strategy (8 NeuronCores, tensor-parallel over vocab):
  - Each core owns a 4000-wide vocab slice of W and B (sharding hint),
    zero-padded to 4096 so weight blocks are exactly 128 columns (the
    tensor engine's fast-weight-load path requires 128-column loads).
  - Host pre-transposes/blocks the operands (fp16) so every DMA is a
    contiguous slab:
      wtb[vb, p, dc, j] = W[v0 + vb*128 + j, dc*128 + p]   (per-core)
      htt = hidden^T [4096, 2048]                           (replicated)
      att = A^T      [4096, 16]                             (replicated)
      btt = B^T slice [16, 4096]                            (per-core)
  - On device (per core): compute logits^T[v, tok] on the PE:
      out[j, t] = sum_dc  wt[128d, 128v].T @ ht[128d, 512t]
                + bt[16r, 128v].T @ ao[16r, 512t]           (LoRA, K=16)
    where ao[r, t] = sum_dc at[128d, 16r].T @ ht[128d, 512t] is computed
    on device first.
  - hidden^T is fp16 so the full 2048 tokens stay SBUF-resident
    (128 KB/partition); W streams through exactly once => PE-bound at
    1 cycle/row: 4096 matmuls x 512 rows ~ 874 us/core.
  - fp16 numerics: products are exact in fp32 PSUM; only the 2^-11 input
    rounding contributes. Measured ~4e-4 absmax relative to output scale.
"""

import numpy as np

import concourse.bass as bass
import concourse.mybir as mybir
import concourse.tile as tile
from concourse import bacc
from concourse.bass_utils import run_bass_kernel_spmd

P = 128
N_TOK = 2048
D = 4096
V = 32000
R = 16
NCORES = 8

VC = V // NCORES          # 4000 real vocab per core
VCP = 4096                # padded vocab per core (32 blocks of 128)
VBS = 128                 # vocab block = psum partition dim = FWL width
VB = VCP // VBS           # 32 vocab blocks
DC = D // P               # 32 contraction chunks
TBS = 512                 # moving free dim per matmul (ISA cap)
TB = N_TOK // TBS         # 4 token blocks

F32 = mybir.dt.float32
F16 = mybir.dt.float16


def build_nc(ht_bufs=DC, wt_bufs=3, out_bufs=4, ps_bufs=6):
    nc = bacc.Bacc(None, target_bir_lowering=False, debug=False)

    wtb = nc.dram_tensor("wtb", [VB, P, DC, VBS], F16, kind="ExternalInput")
    htt = nc.dram_tensor("htt", [D, N_TOK], F16, kind="ExternalInput")
    att = nc.dram_tensor("att", [D, R], F16, kind="ExternalInput")
    btt = nc.dram_tensor("btt", [R, VCP], F16, kind="ExternalInput")
    outt = nc.dram_tensor("outt", [VCP, N_TOK], F32, kind="ExternalOutput")

    att3 = att.rearrange("(c p) r -> p c r", p=P)

    with tile.TileContext(nc) as tc:
        with (
            tc.tile_pool(name="const", bufs=1) as const,
            tc.tile_pool(name="htp", bufs=ht_bufs) as htp,
            tc.tile_pool(name="wtp", bufs=wt_bufs) as wtp,
            tc.tile_pool(name="outp", bufs=out_bufs) as outp,
            tc.tile_pool(name="psp", bufs=ps_bufs, space="PSUM") as psp,
            tc.tile_pool(name="aops", bufs=2, space="PSUM") as aops,
        ):
            at_t = const.tile([P, DC, R], F16, name="at_t")
            nc.sync.dma_start(at_t[:], att3)
            bt_t = const.tile([R, VCP], F16, name="bt_t")
            nc.gpsimd.dma_start(bt_t[:], btt[:, :])
            ao_t = const.tile([R, N_TOK], F16, name="ao_t")

            # prefetch the first weight blocks ahead of the hidden stream
            wt_tiles = {}
            for vb in range(2):
                wt_t = wtp.tile([P, DC, VBS], F16, name="wt_t", tag="wt")
                nc.sync.dma_start(wt_t[:], wtb[vb, :, :, :])
                wt_tiles[vb] = wt_t

            # resident hidden^T: 32 tiles of [128, 2048] fp16
            ht_tiles = []
            for dc in range(DC):
                ht_t = htp.tile([P, N_TOK], F16, name=f"ht_{dc}", tag="ht")
                nc.sync.dma_start(ht_t[:], htt[dc * P:(dc + 1) * P, :])
                ht_tiles.append(ht_t)

            # LoRA activations ao[r, t] (K=4096 accumulation)
            for tb in range(TB):
                pa = aops.tile([R, TBS], F32, name="pa", tag="pa")
                for dc in range(DC):
                    nc.tensor.matmul(
                        pa[:],
                        at_t[:, dc, :],
                        ht_tiles[dc][:, tb * TBS:(tb + 1) * TBS],
                        start=(dc == 0),
                        stop=(dc == DC - 1),
                    )
                nc.vector.tensor_copy(
                    ao_t[:, tb * TBS:(tb + 1) * TBS], pa[:]
                )

            for vb in range(VB):
                if vb in wt_tiles:
                    wt_t = wt_tiles.pop(vb)
                else:
                    wt_t = wtp.tile([P, DC, VBS], F16, name="wt_t", tag="wt")
                    nc.sync.dma_start(wt_t[:], wtb[vb, :, :, :])

                pss = [
                    psp.tile([VBS, TBS], F32, name=f"ps{tb}", tag="ps")
                    for tb in range(TB)
                ]
                for dc in range(DC):
                    for tb in range(TB):
                        nc.tensor.matmul(
                            pss[tb][:],
                            wt_t[:, dc, :],
                            ht_tiles[dc][:, tb * TBS:(tb + 1) * TBS],
                            start=(dc == 0),
                            stop=False,
                        )
                for tb in range(TB):
                    ts0 = tb * TBS
                    # fold LoRA correction into the same psum group
                    nc.tensor.matmul(
                        pss[tb][:],
                        bt_t[:, vb * VBS:(vb + 1) * VBS],
                        ao_t[:, ts0:ts0 + TBS],
                        start=False,
                        stop=True,
                    )
                    ot = outp.tile([VBS, TBS], F32, name="ot", tag="ot")
                    nc.vector.tensor_copy(ot[:], pss[tb][:])
                    nc.scalar.dma_start(
                        outt[vb * VBS:(vb + 1) * VBS, ts0:ts0 + TBS], ot[:]
                    )
    nc.compile()
    return nc


def _prep_inputs(hidden_states, weight, lora_A, lora_B):
    w = np.asarray(weight, dtype=np.float16).reshape(NCORES, VC, D)
    wp = np.zeros((NCORES, VCP, D), dtype=np.float16)
    wp[:, :VC, :] = w
    # [core, vb, j, dc, p] -> [core, vb, p, dc, j]
    wtb_all = np.ascontiguousarray(
        wp.reshape(NCORES, VB, VBS, DC, P).transpose(0, 1, 4, 3, 2)
    )
    htt = np.ascontiguousarray(np.asarray(hidden_states, dtype=np.float16).T)
    att = np.ascontiguousarray(np.asarray(lora_A, dtype=np.float16).T)
    b = np.asarray(lora_B, dtype=np.float16).reshape(NCORES, VC, R)
    bp = np.zeros((NCORES, VCP, R), dtype=np.float16)
    bp[:, :VC, :] = b
    btt_all = np.ascontiguousarray(bp.transpose(0, 2, 1))
    return [
        {
            "wtb": wtb_all[c],
            "htt": htt,
            "att": att,
            "btt": btt_all[c],
        }
        for c in range(NCORES)
    ]


def run(hidden_states, weight, lora_A, lora_B, trace=False, **run_kwargs):
    in_maps = _prep_inputs(hidden_states, weight, lora_A, lora_B)
    nc = build_nc()
    res = run_bass_kernel_spmd(
        nc, in_maps, core_ids=list(range(NCORES)), trace=trace, **run_kwargs
    )
    out = np.empty((N_TOK, V), dtype=np.float32)
    for c in range(NCORES):
        out[:, c * VC:(c + 1) * VC] = res.results[c]["outt"][:VC].T
    return out, res


def kernel(hidden_states, weight, lora_A, lora_B):
    out, _ = run(hidden_states, weight, lora_A, lora_B, trace=False)
    return out


# revision 8
# speedup vs baseline: 1.1847x; 1.1847x over previous
"""Trainium2 Bass kernel for ParallelLMHeadWithLoRA.

logits = hidden @ W^T + (hidden @ A^T) @ B^T
  hidden [2048, 4096] f32, W [32000, 4096] f32, A [16, 4096], B [32000, 16]

Strategy (8 NeuronCores, tensor-parallel over vocab):
  - Each core owns a 4000-wide vocab slice of W and B (sharding hint),
    zero-padded to 4096 so weight blocks are exactly 128 columns (the
    tensor engine's fast-weight-load path requires 128-column loads).
  - Host pre-transposes/blocks the operands (fp16) so every DMA is a
    contiguous slab:
      wtb[vb, p, dc, j] = W[v0 + vb*128 + j, dc*128 + p]   (per-core)
      htt = hidden^T [4096, 2048]                           (replicated)
      att = A^T      [4096, 16]                             (replicated)
      btt = B^T slice [16, 4096]                            (per-core)
  - On device (per core): compute logits^T[v, tok] on the PE:
      out[j, t] = sum_dc  wt[128d, 128v].T @ ht[128d, 512t]
                + bt[16r, 128v].T @ ao[16r, 512t]           (LoRA, K=16)
    where ao[r, t] = sum_dc at[128d, 16r].T @ ht[128d, 512t] is computed
    on device first.
  - hidden^T is fp16 so the full 2048 tokens stay SBUF-resident
    (128 KB/partition); W streams through exactly once => PE-bound at
    1 cycle/row: 4096 matmuls x 512 rows ~ 874 us/core.
  - fp16 numerics: products are exact in fp32 PSUM; only the 2^-11 input
    rounding contributes. Measured ~4e-4 absmax relative to output scale.
"""

import numpy as np

import concourse.bass as bass
import concourse.mybir as mybir
import concourse.tile as tile
from concourse import bacc
from concourse.bass_utils import run_bass_kernel_spmd

P = 128
N_TOK = 2048
D = 4096
V = 32000
R = 16
NCORES = 8

VC = V // NCORES          # 4000 vocab per core
VCP = VC                  # no padding needed at 125-wide blocks
VBS = 125                 # vocab block (psum partition dim)
VB = VCP // VBS           # 32 vocab blocks
DC = D // P               # 32 contraction chunks
TBS = 512                 # moving free dim per matmul (ISA cap)
TB = N_TOK // TBS         # 4 token blocks

F32 = mybir.dt.float32
F16 = mybir.dt.float16


def build_nc(ht_bufs=2 * DC, wt_bufs=3, out_bufs=4, ps_bufs=6):
    nc = bacc.Bacc(None, target_bir_lowering=False, debug=False)

    wtb = nc.dram_tensor("wtb", [VB, P, DC, VBS], F16, kind="ExternalInput")
    htt = nc.dram_tensor("htt", [D, N_TOK], F16, kind="ExternalInput")
    att = nc.dram_tensor("att", [D, R], F16, kind="ExternalInput")
    btt = nc.dram_tensor("btt", [R, VCP], F16, kind="ExternalInput")
    outt = nc.dram_tensor("outt", [VCP, N_TOK], F32, kind="ExternalOutput")

    att3 = att.rearrange("(c p) r -> p c r", p=P)

    with tile.TileContext(nc) as tc:
        with (
            tc.tile_pool(name="const", bufs=1) as const,
            tc.tile_pool(name="htp", bufs=ht_bufs) as htp,
            tc.tile_pool(name="wtp", bufs=wt_bufs) as wtp,
            tc.tile_pool(name="outp", bufs=out_bufs) as outp,
            tc.tile_pool(name="psp", bufs=ps_bufs, space="PSUM") as psp,
            tc.tile_pool(name="aops", bufs=2, space="PSUM") as aops,
        ):
            at_t = const.tile([P, DC, R], F16, name="at_t")
            nc.sync.dma_start(at_t[:], att3)
            bt_t = const.tile([R, VCP], F16, name="bt_t")
            nc.gpsimd.dma_start(bt_t[:], btt[:, :])
            ao_t = const.tile([R, N_TOK], F16, name="ao_t")

            # prefetch the first weight blocks ahead of the hidden stream
            wt_tiles = {}
            for vb in range(2):
                wt_t = wtp.tile([P, DC, VBS], F16, name="wt_t", tag="wt")
                nc.sync.dma_start(wt_t[:], wtb[vb, :, :, :])
                wt_tiles[vb] = wt_t

            # resident hidden^T: 64 tiles of [128, 1024] fp16 (full 2048 tokens)
            ht_tiles = {}
            for dc in range(DC):
                for g in range(2):
                    ht_t = htp.tile([P, N_TOK // 2], F16,
                                    name=f"ht_{dc}_{g}", tag="ht")
                    nc.sync.dma_start(
                        ht_t[:],
                        htt[dc * P:(dc + 1) * P,
                            g * (N_TOK // 2):(g + 1) * (N_TOK // 2)],
                    )
                    ht_tiles[(dc, g)] = ht_t

            def ht_slice(dc, tb):
                g, r = divmod(tb, 2)
                return ht_tiles[(dc, g)][:, r * TBS:(r + 1) * TBS]

            # LoRA activations ao[r, t] (K=4096 accumulation)
            for tb in range(TB):
                pa = aops.tile([R, TBS], F32, name="pa", tag="pa")
                for dc in range(DC):
                    nc.tensor.matmul(
                        pa[:],
                        at_t[:, dc, :],
                        ht_slice(dc, tb),
                        start=(dc == 0),
                        stop=(dc == DC - 1),
                    )
                nc.vector.tensor_copy(
                    ao_t[:, tb * TBS:(tb + 1) * TBS], pa[:]
                )

            for vb in range(VB):
                if vb in wt_tiles:
                    wt_t = wt_tiles.pop(vb)
                else:
                    wt_t = wtp.tile([P, DC, VBS], F16, name="wt_t", tag="wt")
                    nc.sync.dma_start(wt_t[:], wtb[vb, :, :, :])

                pss = [
                    psp.tile([VBS, TBS], F32, name=f"ps{tb}", tag="ps")
                    for tb in range(TB)
                ]
                for dc in range(DC):
                    for tb in range(TB):
                        nc.tensor.matmul(
                            pss[tb][:],
                            wt_t[:, dc, :],
                            ht_slice(dc, tb),
                            start=(dc == 0),
                            stop=False,
                        )
                for tb in range(TB):
                    ts0 = tb * TBS
                    # fold LoRA correction into the same psum group
                    nc.tensor.matmul(
                        pss[tb][:],
                        bt_t[:, vb * VBS:(vb + 1) * VBS],
                        ao_t[:, ts0:ts0 + TBS],
                        start=False,
                        stop=True,
                    )
                    ot = outp.tile([VBS, TBS], F32, name="ot", tag="ot")
                    nc.vector.tensor_copy(ot[:], pss[tb][:])
                    nc.scalar.dma_start(
                        outt[vb * VBS:(vb + 1) * VBS, ts0:ts0 + TBS], ot[:]
                    )
    nc.compile()
    return nc


def _prep_inputs(hidden_states, weight, lora_A, lora_B):
    w = np.asarray(weight, dtype=np.float16)
    # [core, vb, j, dc, p] -> [core, vb, p, dc, j]
    wtb_all = np.ascontiguousarray(
        w.reshape(NCORES, VB, VBS, DC, P).transpose(0, 1, 4, 3, 2)
    )
    htt = np.ascontiguousarray(np.asarray(hidden_states, dtype=np.float16).T)
    att = np.ascontiguousarray(np.asarray(lora_A, dtype=np.float16).T)
    btt_all = np.ascontiguousarray(
        np.asarray(lora_B, dtype=np.float16).reshape(NCORES, VC, R)
        .transpose(0, 2, 1)
    )
    return [
        {
            "wtb": wtb_all[c],
            "htt": htt,
            "att": att,
            "btt": btt_all[c],
        }
        for c in range(NCORES)
    ]


def run(hidden_states, weight, lora_A, lora_B, trace=False, **run_kwargs):
    in_maps = _prep_inputs(hidden_states, weight, lora_A, lora_B)
    nc = build_nc()
    res = run_bass_kernel_spmd(
        nc, in_maps, core_ids=list(range(NCORES)), trace=trace, **run_kwargs
    )
    out = np.empty((N_TOK, V), dtype=np.float32)
    for c in range(NCORES):
        out[:, c * VC:(c + 1) * VC] = res.results[c]["outt"].T
    return out, res


def kernel(hidden_states, weight, lora_A, lora_B):
    out, _ = run(hidden_states, weight, lora_A, lora_B, trace=False)
    return out


# revision 9
# speedup vs baseline: 1.1896x; 1.0042x over previous
"""Trainium2 Bass kernel for ParallelLMHeadWithLoRA.

logits = hidden @ W^T + (hidden @ A^T) @ B^T
  hidden [2048, 4096] f32, W [32000, 4096] f32, A [16, 4096], B [32000, 16]

Strategy (8 NeuronCores, tensor-parallel over vocab):
  - Each core owns a 4000-wide vocab slice of W and B (sharding hint),
    zero-padded to 4096 so weight blocks are exactly 128 columns (the
    tensor engine's fast-weight-load path requires 128-column loads).
  - Host pre-transposes/blocks the operands (fp16) so every DMA is a
    contiguous slab:
      wtb[vb, p, dc, j] = W[v0 + vb*128 + j, dc*128 + p]   (per-core)
      htt = hidden^T [4096, 2048]                           (replicated)
      att = A^T      [4096, 16]                             (replicated)
      btt = B^T slice [16, 4096]                            (per-core)
  - On device (per core): compute logits^T[v, tok] on the PE:
      out[j, t] = sum_dc  wt[128d, 128v].T @ ht[128d, 512t]
                + bt[16r, 128v].T @ ao[16r, 512t]           (LoRA, K=16)
    where ao[r, t] = sum_dc at[128d, 16r].T @ ht[128d, 512t] is computed
    on device first.
  - hidden^T is fp16 so the full 2048 tokens stay SBUF-resident
    (128 KB/partition); W streams through exactly once => PE-bound at
    1 cycle/row: 4096 matmuls x 512 rows ~ 874 us/core.
  - fp16 numerics: products are exact in fp32 PSUM; only the 2^-11 input
    rounding contributes. Measured ~4e-4 absmax relative to output scale.
"""

import numpy as np

import concourse.bass as bass
import concourse.mybir as mybir
import concourse.tile as tile
from concourse import bacc
from concourse.bass_utils import run_bass_kernel_spmd

P = 128
N_TOK = 2048
D = 4096
V = 32000
R = 16
NCORES = 8

VC = V // NCORES          # 4000 vocab per core
VCP = VC                  # no padding needed at 125-wide blocks
VBS = 125                 # vocab block (psum partition dim)
VB = VCP // VBS           # 32 vocab blocks
DC = D // P               # 32 contraction chunks
TBS = 512                 # moving free dim per matmul (ISA cap)
TB = N_TOK // TBS         # 4 token blocks

F32 = mybir.dt.float32
F16 = mybir.dt.float16


def build_nc(ht_bufs=2 * DC, wt_bufs=3, out_bufs=4, ps_bufs=6):
    nc = bacc.Bacc(None, target_bir_lowering=False, debug=False)

    wtb = nc.dram_tensor("wtb", [VB, P, DC, VBS], F16, kind="ExternalInput")
    htt = nc.dram_tensor("htt", [D, N_TOK], F16, kind="ExternalInput")
    att = nc.dram_tensor("att", [P, DC, R], F16, kind="ExternalInput")
    btt = nc.dram_tensor("btt", [R, VCP], F16, kind="ExternalInput")
    outt = nc.dram_tensor("outt", [VCP, N_TOK], F32, kind="ExternalOutput")

    with tile.TileContext(nc) as tc:
        with (
            tc.tile_pool(name="const", bufs=1) as const,
            tc.tile_pool(name="htp", bufs=ht_bufs) as htp,
            tc.tile_pool(name="wtp", bufs=wt_bufs) as wtp,
            tc.tile_pool(name="outp", bufs=out_bufs) as outp,
            tc.tile_pool(name="psp", bufs=ps_bufs, space="PSUM") as psp,
            tc.tile_pool(name="aops", bufs=2, space="PSUM") as aops,
        ):
            # prefetch the first weight blocks ahead of the hidden stream
            wt_tiles = {}
            for vb in range(2):
                wt_t = wtp.tile([P, DC, VBS], F16, name="wt_t", tag="wt")
                nc.sync.dma_start(wt_t[:], wtb[vb, :, :, :])
                wt_tiles[vb] = wt_t

            at_t = const.tile([P, DC, R], F16, name="at_t")
            nc.gpsimd.dma_start(at_t[:], att[:, :, :])
            bt_t = const.tile([R, VCP], F16, name="bt_t")
            nc.gpsimd.dma_start(bt_t[:], btt[:, :])
            ao_t = const.tile([R, N_TOK], F16, name="ao_t")

            # resident hidden^T: 64 tiles of [128, 1024] fp16 (full 2048 tokens)
            ht_tiles = {}
            for dc in range(DC):
                for g in range(2):
                    ht_t = htp.tile([P, N_TOK // 2], F16,
                                    name=f"ht_{dc}_{g}", tag="ht")
                    nc.sync.dma_start(
                        ht_t[:],
                        htt[dc * P:(dc + 1) * P,
                            g * (N_TOK // 2):(g + 1) * (N_TOK // 2)],
                    )
                    ht_tiles[(dc, g)] = ht_t

            def ht_slice(dc, tb):
                g, r = divmod(tb, 2)
                return ht_tiles[(dc, g)][:, r * TBS:(r + 1) * TBS]

            # LoRA activations ao[r, t] (K=4096 accumulation)
            for tb in range(TB):
                pa = aops.tile([R, TBS], F32, name="pa", tag="pa")
                for dc in range(DC):
                    nc.tensor.matmul(
                        pa[:],
                        at_t[:, dc, :],
                        ht_slice(dc, tb),
                        start=(dc == 0),
                        stop=(dc == DC - 1),
                    )
                nc.vector.tensor_copy(
                    ao_t[:, tb * TBS:(tb + 1) * TBS], pa[:]
                )

            for vb in range(VB):
                if vb in wt_tiles:
                    wt_t = wt_tiles.pop(vb)
                else:
                    wt_t = wtp.tile([P, DC, VBS], F16, name="wt_t", tag="wt")
                    nc.sync.dma_start(wt_t[:], wtb[vb, :, :, :])

                pss = [
                    psp.tile([VBS, TBS], F32, name=f"ps{tb}", tag="ps")
                    for tb in range(TB)
                ]
                for dc in range(DC):
                    for tb in range(TB):
                        nc.tensor.matmul(
                            pss[tb][:],
                            wt_t[:, dc, :],
                            ht_slice(dc, tb),
                            start=(dc == 0),
                            stop=False,
                        )
                for tb in range(TB):
                    ts0 = tb * TBS
                    # fold LoRA correction into the same psum group
                    nc.tensor.matmul(
                        pss[tb][:],
                        bt_t[:, vb * VBS:(vb + 1) * VBS],
                        ao_t[:, ts0:ts0 + TBS],
                        start=False,
                        stop=True,
                    )
                    ot = outp.tile([VBS, TBS], F32, name="ot", tag="ot")
                    nc.vector.tensor_copy(ot[:], pss[tb][:])
                    nc.scalar.dma_start(
                        outt[vb * VBS:(vb + 1) * VBS, ts0:ts0 + TBS], ot[:]
                    )
    nc.compile()
    return nc


def _prep_inputs(hidden_states, weight, lora_A, lora_B):
    w = np.asarray(weight, dtype=np.float16)
    # [core, vb, j, dc, p] -> [core, vb, p, dc, j]
    wtb_all = np.ascontiguousarray(
        w.reshape(NCORES, VB, VBS, DC, P).transpose(0, 1, 4, 3, 2)
    )
    htt = np.ascontiguousarray(np.asarray(hidden_states, dtype=np.float16).T)
    att = np.ascontiguousarray(
        np.asarray(lora_A, dtype=np.float16).T.reshape(DC, P, R).transpose(1, 0, 2)
    )
    btt_all = np.ascontiguousarray(
        np.asarray(lora_B, dtype=np.float16).reshape(NCORES, VC, R)
        .transpose(0, 2, 1)
    )
    return [
        {
            "wtb": wtb_all[c],
            "htt": htt,
            "att": att,
            "btt": btt_all[c],
        }
        for c in range(NCORES)
    ]


def run(hidden_states, weight, lora_A, lora_B, trace=False, **run_kwargs):
    in_maps = _prep_inputs(hidden_states, weight, lora_A, lora_B)
    nc = build_nc()
    res = run_bass_kernel_spmd(
        nc, in_maps, core_ids=list(range(NCORES)), trace=trace, **run_kwargs
    )
    out = np.empty((N_TOK, V), dtype=np.float32)
    for c in range(NCORES):
        out[:, c * VC:(c + 1) * VC] = res.results[c]["outt"].T
    return out, res


def kernel(hidden_states, weight, lora_A, lora_B):
    out, _ = run(hidden_states, weight, lora_A, lora_B, trace=False)
    return out
